# revision 6
# baseline (speedup 1.0000x reference)
"""Trainium2 Bass kernel for BlittingStrokeModel (AA polyline rasterization).

Reference semantics: for each batch item, rasterize 16 AA line segments
(trajectory knots) onto a zero canvas via a point-to-segment distance field:
    dist = point-to-segment distance
    cov  = clip(line_width + 0.5 - dist, 0, 1)
    out  = max over segments, broadcast to 3 channels.

Device formulation (exact up to the reference's 1e-8/1e-12 epsilons), using
the normalized segment parameter t so the clamp bounds are the constants
0 and 1 (immediate-only ops, runnable on GPSIMD):
    t   = (dx*x + dy*y - c0) / dd2          # dd2 = dx^2+dy^2
    et  = relu(|t - 1/2| - 1/2)             # distance of t outside [0,1]
    cap = (et * sqrt(dd2))^2                # along-line excess^2
    P   = (dy*x - dx*y + cP) / sqrt(dd2)    # perpendicular line distance
    dist^2 = cap + P^2
    M   = min over segments of dist^2
    cov = clip(L + 0.5 - sqrt(M), 0, 1)
Max over segments of cov == cov(min dist) since cov is monotone in dist.

Sharding: data-parallel over batch, one image per NeuronCore (8 cores).
The output does not depend on the image *values*, so images never touch the
device; only tiny per-segment coefficient tables are uploaded.
"""

import numpy as np
from contextlib import ExitStack

B, C, H, W = 8, 3, 512, 512
K = 17
NSEG = K - 1
P = 128
NSTRIPE = H // P  # 4

# per-seg uniform scalar columns in `cs`
CS_DXA, CS_AP, CS_SQD = 0, 1, 2
CS_PER_SEG = 3
CS_THR = CS_PER_SEG * NSEG
CS_COLS = CS_THR + 1

# MODE: "no_pe"  — vector does the cap+p2 add (no TensorE use)
#       "pe_f32r" — TensorE identity-matmuls accumulate cap+p2 in PSUM
MODE = "pe_f32r"

_state = {}


def _build_program(mode=MODE):
    import concourse.bass as bass
    import concourse.tile as tile
    from concourse import bacc, mybir

    dt = mybir.dt
    op = mybir.AluOpType
    af = mybir.ActivationFunctionType

    nc = bacc.Bacc(
        "TRN2", target_bir_lowering=False, debug=False, num_devices=8
    )
    xt_d = nc.dram_tensor("xt", [P, W], dt.float32, kind="ExternalInput").ap()
    cs_d = nc.dram_tensor("cs", [P, CS_COLS], dt.float32, kind="ExternalInput").ap()
    cdt_d = nc.dram_tensor("cdt", [P, NSTRIPE * NSEG], dt.float32, kind="ExternalInput").ap()
    cbp_d = nc.dram_tensor("cbp", [P, NSTRIPE * NSEG], dt.float32, kind="ExternalInput").ap()
    ident_d = nc.dram_tensor("ident", [P, P], dt.float32r, kind="ExternalInput").ap()
    out_d = nc.dram_tensor("out", [C, H, W], dt.float32, kind="ExternalOutput").ap()

    with tile.TileContext(nc) as tc, ExitStack() as ctx:
        const = ctx.enter_context(tc.tile_pool(name="const", bufs=1))
        xt = const.tile_from(xt_d)
        cs = const.tile_from(cs_d)
        cdt = const.tile_from(cdt_d)
        cbp = const.tile_from(cbp_d)
        ident = (
            const.tile_from(ident_d, name="ident") if mode == "pe_f32r" else None
        )

        work = ctx.enter_context(tc.tile_pool(name="work", bufs=3))
        mpool = ctx.enter_context(tc.tile_pool(name="m", bufs=2))
        opool = ctx.enter_context(tc.tile_pool(name="o", bufs=3))
        if mode == "pe_f32r":
            psum = ctx.enter_context(tc.tile_pool(name="ps", bufs=4, space="PSUM"))

        def seg_col(s, which):
            c = s * CS_PER_SEG + which
            return cs[:, c : c + 1]

        sq_dtype = dt.float32r if mode == "pe_f32r" else dt.float32
        for T in range(NSTRIPE):
            M = mpool.tile([P, W], dt.float32, tag="M")
            for s in range(NSEG):
                col = T * NSEG + s
                # At = |t - 1/2| = |x*dxa + (cdt - 1/2)|   [scalar ACT]
                At = work.tile([P, W], dt.float32, tag="At")
                nc.scalar.activation(
                    At[:], xt[:], af.Abs,
                    bias=cdt[:, col : col + 1], scale=seg_col(s, CS_DXA),
                )
                # Et = relu(At - 1/2)   [vector ts]
                Et = work.tile([P, W], dt.float32, tag="Et")
                nc.vector.tensor_scalar(
                    Et[:], At[:], 0.5, 0.0, op0=op.subtract, op1=op.max
                )
                # Pp = aP*x + bP   [vector ts]
                Pp = work.tile([P, W], dt.float32, tag="Pp")
                nc.vector.tensor_scalar(
                    Pp[:], xt[:], seg_col(s, CS_AP), cbp[:, col : col + 1],
                    op0=op.mult, op1=op.add,
                )
                # cap = (Et*sqd)^2   [scalar ACT]
                cap = work.tile([P, W], sq_dtype, tag="cap")
                nc.scalar.activation(cap[:], Et[:], af.Square, scale=seg_col(s, CS_SQD))
                # p2s = Pp^2   [gpsimd tensor_tensor]
                p2s = work.tile([P, W], sq_dtype, tag="p2s")
                nc.gpsimd.tensor_tensor(p2s[:], Pp[:], Pp[:], op=op.mult)

                if mode == "pe_f32r":
                    ps = psum.tile([P, W], dt.float32, tag="ps")
                    nc.tensor.matmul(
                        ps[:], ident[:], cap[:], start=True, stop=False
                    )
                    nc.tensor.matmul(
                        ps[:], ident[:], p2s[:], start=False, stop=True
                    )
                    d2 = ps
                else:
                    d2 = work.tile([P, W], dt.float32, tag="d2")
                    nc.vector.tensor_tensor(d2[:], cap[:], p2s[:], op=op.add)

                if s == 0:
                    nc.vector.tensor_copy(M[:], d2[:])
                else:
                    nc.vector.tensor_tensor(M[:], M[:], d2[:], op=op.min)

            # dist = sqrt(M); cov = clip(thr - dist, 0, 1)
            dist = opool.tile([P, W], dt.float32, tag="dist")
            nc.scalar.activation(dist[:], M[:], af.Sqrt)
            cov1 = opool.tile([P, W], dt.float32, tag="cov1")
            nc.scalar.activation(
                cov1[:], dist[:], af.Identity,
                bias=cs[:, CS_THR : CS_THR + 1], scale=-1.0,
            )
            cov = opool.tile([P, W], dt.float32, tag="cov")
            nc.vector.tensor_scalar(
                cov[:], cov1[:], 0.0, 1.0, op0=op.max, op1=op.min
            )
            for c in range(C):
                nc.sync.dma_start(out_d[c, T * P : (T + 1) * P, :], cov[:])

    nc.compile()
    return nc


def _prep_inputs(trajectories, line_width):
    """Host-side per-segment coefficient tables (numpy, float64 -> float32)."""
    thr = float(np.asarray(line_width).item()) + 0.5
    xt = np.broadcast_to(
        np.arange(W, dtype=np.float64), (P, W)
    ).astype(np.float32)
    ident = np.eye(P, dtype=np.float32)
    xy = np.asarray(trajectories, dtype=np.float64)[:, :, 1:3]  # [B, K, 2]
    yv = np.arange(H, dtype=np.float64).reshape(NSTRIPE, P)  # y = T*128 + p

    in_maps = []
    for b in range(xy.shape[0]):
        p0, p1 = xy[b, :-1], xy[b, 1:]
        d = p1 - p0
        # degenerate-segment guard (measure-zero with random inputs)
        degen = (d[:, 0] ** 2 + d[:, 1] ** 2) < 1e-12
        d[degen, 0] = 1e-6
        dx, dy = d[:, 0], d[:, 1]
        p0x, p0y = p0[:, 0], p0[:, 1]
        dd2 = dx * dx + dy * dy
        sq = 1.0 / np.sqrt(dd2)
        c0 = dx * p0x + dy * p0y
        cP = dx * p0y - dy * p0x

        # [T, p, s] -> [p, T*NSEG+s]; fold the -1/2 of |t - 1/2| into the bias
        cdt = (dy[None, None, :] * yv[:, :, None] - c0[None, None, :]) / dd2[None, None, :] - 0.5
        cdt = np.transpose(cdt, (1, 0, 2)).reshape(P, NSTRIPE * NSEG)
        cbp = (-dx[None, None, :] * yv[:, :, None] + cP[None, None, :]) * sq[None, None, :]
        cbp = np.transpose(cbp, (1, 0, 2)).reshape(P, NSTRIPE * NSEG)

        cs = np.zeros((P, CS_COLS), dtype=np.float64)
        cs[:, CS_DXA : CS_PER_SEG * NSEG : CS_PER_SEG] = dx / dd2
        cs[:, CS_AP : CS_PER_SEG * NSEG : CS_PER_SEG] = dy * sq
        cs[:, CS_SQD : CS_PER_SEG * NSEG : CS_PER_SEG] = np.sqrt(dd2)
        cs[:, CS_THR] = thr

        in_maps.append(
            {
                "xt": xt,
                "cs": cs.astype(np.float32),
                "cdt": cdt.astype(np.float32),
                "cbp": cbp.astype(np.float32),
                "ident": ident,
            }
        )
    return in_maps


def kernel(**inputs):
    from concourse.bass_utils import run_bass_kernel_spmd

    images = np.asarray(inputs["images"])
    trajectories = np.asarray(inputs["trajectories"])
    line_width = inputs["line_width"]
    assert images.shape == (B, C, H, W), images.shape

    if "nc" not in _state:
        _state["nc"] = _build_program()
    nc = _state["nc"]

    in_maps = _prep_inputs(trajectories, line_width)
    res = run_bass_kernel_spmd(nc, in_maps, list(range(B))).results
    out = np.stack([res[i]["out"] for i in range(B)], axis=0)
    return out.astype(np.float32)


if __name__ == "__main__":
    rng = np.random.default_rng(0)
    ins = {
        "images": rng.standard_normal((B, C, H, W)).astype(np.float32),
        "trajectories": np.concatenate(
            [
                np.broadcast_to(np.linspace(0, 1, K, dtype=np.float32), (B, K))[..., None],
                rng.uniform(0, W - 1, (B, K, 2)).astype(np.float32),
                np.ones((B, K, 1), np.float32),
            ],
            axis=-1,
        ),
        "line_width": 3,
    }
    out = kernel(**ins)
    print(out.shape, out.dtype, out.min(), out.max())


# revision 11
# speedup vs baseline: 1.4402x; 1.4402x over previous
"""Trainium2 Bass kernel for BlittingStrokeModel (AA polyline rasterization).

Reference semantics: for each batch item, rasterize 16 AA line segments
(trajectory knots) onto a zero canvas via a point-to-segment distance field:
    dist = point-to-segment distance
    cov  = clip(line_width + 0.5 - dist, 0, 1)
    out  = max over segments, broadcast to 3 channels.

Device formulation (exact up to the reference's 1e-8/1e-12 epsilons). With
s = 1/sqrt(dd2), dd2 = dx^2+dy^2, dn2 = dd2/2:
    w   = (dx*x + dy*y - c0 - dn2) * s        # scaled, recentred dot product
    E   = relu(|w| - dn2*s)                   # segment-clamp excess / sqrt(dd2)
    Pp  = (dy*x - dx*y + cP) * s              # perpendicular line distance
    dist^2 = Pp^2 + E^2
    M   = min over segments of dist^2
    cov = clip(L + 0.5 - sqrt(M), 0, 1)
Max over segments of cov == cov(min dist) since cov is monotone in dist.

Engine split per (segment, 128-row stripe):
    ACT:   At = Abs(x*s0 + bias)              (plane + |.|)
    DVE:   E  = tensor_scalar(At, -dn2s, max 0)
    DVE:   d2 = custom fused op  sq(x*aP + bP) + sq(E)   -> directly min'd
    GPSIMD: M = min(M, d2)
The custom DVE op (P2SQ_ADD_SQ) is registered at runtime into
concourse.dve_ops.OPS, so this file is self-contained.

Sharding: data-parallel over batch, one image per NeuronCore (8 cores).
The output does not depend on the image *values*, so images never touch the
device; only tiny per-segment coefficient tables are uploaded.
"""

import numpy as np
from contextlib import ExitStack

B, C, H, W = 8, 3, 512, 512
K = 17
NSEG = K - 1
P = 128
NSTRIPE = H // P  # 4

# per-seg uniform scalar columns in `cs`
CS_DXS, CS_AP, CS_DN2S = 0, 1, 2
CS_PER_SEG = 3
CS_THR = CS_PER_SEG * NSEG
CS_COLS = CS_THR + 1

_state = {}


def _register_dve_op(name, spec):
    import concourse.dve_ops as dve_ops
    from concourse.dve_ops import DveOp, OPS, _SUB_OPCODE_FOR_NAME, _CUSTOM_DVE_ROW_BASE
    from concourse.dve_spec import lower, _has_src1
    from concourse.dve_uop import DveOpSpec
    from concourse.dve_table_gen import dve_ver_for

    if name in _SUB_OPCODE_FOR_NAME:
        return next(o for o in OPS if o.name == name)
    row = _CUSTOM_DVE_ROW_BASE + len(OPS)
    assert row < 0x20
    _SUB_OPCODE_FOR_NAME[name] = row
    ver = dve_ver_for("TRN2")
    tmp = DveOpSpec(
        name=name, opcode=row, uops=lower(spec, ver=ver), rd1_en=_has_src1(spec)
    )
    op = DveOp(name, spec, subdim=False, uops_sha={ver: tmp.sha(ver)})
    OPS.append(op)
    dve_ops.CUSTOM_DVE_SPECS[name] = spec
    return op


def _get_dve_ops():
    """Register (once) the two fused DVE ops:
      D2MIN: out = min((Idx*s0 + s1)^2 + Src0^2, Src1)
      D2:    out = (Idx*s0 + s1)^2 + Src0^2        (first segment, no min)
    Idx is the DVE free-dim index generator == the x coordinate, so the
    perpendicular-plane term costs no tensor input and Src1 can carry the
    running minimum."""
    if "ops" in _state:
        return _state["ops"]
    from concourse.dve_spec import Spec, Src0, Src1, C0, C1, sq, minn, Idx

    def _idx(in0):
        return np.arange(in0.shape[-1], dtype=np.float32)[None, :]

    d2min = _register_dve_op(
        "STROKE_D2MIN_ANT",
        Spec(
            body=minn(sq(Idx * C0 + C1) + sq(Src0), Src1),
            reference=lambda in0, in1, s0, s1, imm2: np.minimum(
                (_idx(in0) * s0 + s1) ** 2 + in0.astype(np.float32) ** 2, in1
            ).astype(np.float32),
        ),
    )
    d2first = _register_dve_op(
        "STROKE_D2_ANT",
        Spec(
            body=sq(Idx * C0 + C1) + sq(Src0),
            reference=lambda in0, in1, s0, s1, imm2: (
                (_idx(in0) * s0 + s1) ** 2 + in0.astype(np.float32) ** 2
            ).astype(np.float32),
        ),
    )
    _state["ops"] = (d2min, d2first)
    return _state["ops"]


def _build_program():
    import concourse.bass as bass
    import concourse.tile as tile
    from concourse import bacc, mybir

    dt = mybir.dt
    op = mybir.AluOpType
    af = mybir.ActivationFunctionType
    d2min_op, d2first_op = _get_dve_ops()

    nc = bacc.Bacc(
        "TRN2", target_bir_lowering=False, debug=False, num_devices=8
    )
    xt_d = nc.dram_tensor("xt", [P, W], dt.float32, kind="ExternalInput").ap()
    cs_d = nc.dram_tensor("cs", [P, CS_COLS], dt.float32, kind="ExternalInput").ap()
    cdw_d = nc.dram_tensor("cdw", [P, NSTRIPE * NSEG], dt.float32, kind="ExternalInput").ap()
    cbp_d = nc.dram_tensor("cbp", [P, NSTRIPE * NSEG], dt.float32, kind="ExternalInput").ap()
    out_d = nc.dram_tensor("out", [C, H, W], dt.float32, kind="ExternalOutput").ap()

    with tile.TileContext(nc) as tc, ExitStack() as ctx:
        const = ctx.enter_context(tc.tile_pool(name="const", bufs=1))
        xt = const.tile_from(xt_d)
        cs = const.tile_from(cs_d)
        cdw = const.tile_from(cdw_d)
        cbp = const.tile_from(cbp_d)

        work = ctx.enter_context(tc.tile_pool(name="work", bufs=4))
        mpool = ctx.enter_context(tc.tile_pool(name="m", bufs=3))
        opool = ctx.enter_context(tc.tile_pool(name="o", bufs=3))

        def seg_col(s, which):
            c = s * CS_PER_SEG + which
            return cs[:, c : c + 1]

        for T in range(NSTRIPE):
            M = None
            for s in range(NSEG):
                col = T * NSEG + s
                # At = |(dx*x + dy*y - c0 - dn2) * s|   [scalar ACT]
                At = work.tile([P, W], dt.float32, tag="At")
                nc.scalar.activation(
                    At[:], xt[:], af.Abs,
                    bias=cdw[:, col : col + 1], scale=seg_col(s, CS_DXS),
                )
                # E = relu(At - dn2s)   [vector ts]
                E = work.tile([P, W], dt.float32, tag="E")
                nc.vector.tensor_scalar(
                    E[:], At[:], seg_col(s, CS_DN2S), 0.0,
                    op0=op.subtract, op1=op.max,
                )
                # M' = min((aP*x + bP)^2 + E^2, M)   [one fused custom DVE op]
                Mn = mpool.tile([P, W], dt.float32, tag="M")
                if s == 0:
                    nc.vector._custom_dve(
                        d2first_op, out=Mn[:], in0=E[:],
                        s0=seg_col(s, CS_AP), s1=cbp[:, col : col + 1],
                    )
                else:
                    nc.vector._custom_dve(
                        d2min_op, out=Mn[:], in0=E[:], in1=M[:],
                        s0=seg_col(s, CS_AP), s1=cbp[:, col : col + 1],
                    )
                M = Mn

            # dist = sqrt(M); cov = clip(thr - dist, 0, 1)
            dist = opool.tile([P, W], dt.float32, tag="dist")
            nc.scalar.activation(dist[:], M[:], af.Sqrt)
            cov1 = opool.tile([P, W], dt.float32, tag="cov1")
            nc.scalar.activation(
                cov1[:], dist[:], af.Identity,
                bias=cs[:, CS_THR : CS_THR + 1], scale=-1.0,
            )
            cov = opool.tile([P, W], dt.float32, tag="cov")
            nc.vector.tensor_scalar(
                cov[:], cov1[:], 0.0, 1.0, op0=op.max, op1=op.min
            )
            for c in range(C):
                nc.sync.dma_start(out_d[c, T * P : (T + 1) * P, :], cov[:])

    nc.compile()
    return nc


def _prep_inputs(trajectories, line_width):
    """Host-side per-segment coefficient tables (numpy, float64 -> float32)."""
    thr = float(np.asarray(line_width).item()) + 0.5
    xt = np.broadcast_to(
        np.arange(W, dtype=np.float64), (P, W)
    ).astype(np.float32)
    xy = np.asarray(trajectories, dtype=np.float64)[:, :, 1:3]  # [B, K, 2]
    yv = np.arange(H, dtype=np.float64).reshape(NSTRIPE, P)  # y = T*128 + p

    in_maps = []
    for b in range(xy.shape[0]):
        p0, p1 = xy[b, :-1], xy[b, 1:]
        d = p1 - p0
        # degenerate-segment guard (measure-zero with random inputs)
        degen = (d[:, 0] ** 2 + d[:, 1] ** 2) < 1e-12
        d[degen, 0] = 1e-6
        dx, dy = d[:, 0], d[:, 1]
        p0x, p0y = p0[:, 0], p0[:, 1]
        dd2 = dx * dx + dy * dy
        sq = 1.0 / np.sqrt(dd2)
        dn2 = dd2 / 2.0
        c0 = dx * p0x + dy * p0y
        cP = dx * p0y - dy * p0x

        # [T, p, s] -> [p, T*NSEG+s]
        cdw = (dy[None, None, :] * yv[:, :, None] - (c0 + dn2)[None, None, :]) * sq[None, None, :]
        cdw = np.transpose(cdw, (1, 0, 2)).reshape(P, NSTRIPE * NSEG)
        cbp = (-dx[None, None, :] * yv[:, :, None] + cP[None, None, :]) * sq[None, None, :]
        cbp = np.transpose(cbp, (1, 0, 2)).reshape(P, NSTRIPE * NSEG)

        cs = np.zeros((P, CS_COLS), dtype=np.float64)
        cs[:, CS_DXS : CS_PER_SEG * NSEG : CS_PER_SEG] = dx * sq
        cs[:, CS_AP : CS_PER_SEG * NSEG : CS_PER_SEG] = dy * sq
        cs[:, CS_DN2S : CS_PER_SEG * NSEG : CS_PER_SEG] = dn2 * sq
        cs[:, CS_THR] = thr

        in_maps.append(
            {
                "xt": xt,
                "cs": cs.astype(np.float32),
                "cdw": cdw.astype(np.float32),
                "cbp": cbp.astype(np.float32),
            }
        )
    return in_maps


def kernel(**inputs):
    from concourse.bass_utils import run_bass_kernel_spmd

    images = np.asarray(inputs["images"])
    trajectories = np.asarray(inputs["trajectories"])
    line_width = inputs["line_width"]
    assert images.shape == (B, C, H, W), images.shape

    if "nc" not in _state:
        _state["nc"] = _build_program()
    nc = _state["nc"]

    in_maps = _prep_inputs(trajectories, line_width)
    res = run_bass_kernel_spmd(nc, in_maps, list(range(B))).results
    out = np.stack([res[i]["out"] for i in range(B)], axis=0)
    return out.astype(np.float32)


if __name__ == "__main__":
    rng = np.random.default_rng(0)
    ins = {
        "images": rng.standard_normal((B, C, H, W)).astype(np.float32),
        "trajectories": np.concatenate(
            [
                np.broadcast_to(np.linspace(0, 1, K, dtype=np.float32), (B, K))[..., None],
                rng.uniform(0, W - 1, (B, K, 2)).astype(np.float32),
                np.ones((B, K, 1), np.float32),
            ],
            axis=-1,
        ),
        "line_width": 3,
    }
    out = kernel(**ins)
    print(out.shape, out.dtype, out.min(), out.max())


# revision 15
# speedup vs baseline: 1.7676x; 1.2273x over previous
"""Trainium2 Bass kernel for BlittingStrokeModel (AA polyline rasterization).

Reference semantics: for each batch item, rasterize 16 AA line segments
(trajectory knots) onto a zero canvas via a point-to-segment distance field:
    dist = point-to-segment distance
    cov  = clip(line_width + 0.5 - dist, 0, 1)
    out  = max over segments, broadcast to 3 channels.

Device formulation (exact up to the reference's 1e-8/1e-12 epsilons). With
s = 1/sqrt(dd2), dd2 = dx^2+dy^2, dn2 = dd2/2:
    w   = (dx*x + dy*y - c0 - dn2) * s        # scaled, recentred dot product
    E   = relu(|w| - dn2*s)                   # segment-clamp excess / sqrt(dd2)
    Pp  = (dy*x - dx*y + cP) * s              # perpendicular line distance
    dist^2 = Pp^2 + E^2
    M   = min over segments of dist^2
    cov = clip(L + 0.5 - sqrt(M), 0, 1)
Max over segments of cov == cov(min dist) since cov is monotone in dist.

Engine split per (segment, 128-row stripe):
    ACT:   At = Abs(x*s0 + bias)              (plane + |.|)
    DVE:   E  = tensor_scalar(At, -dn2s, max 0)
    DVE:   d2 = custom fused op  sq(x*aP + bP) + sq(E)   -> directly min'd
    GPSIMD: M = min(M, d2)
The custom DVE op (P2SQ_ADD_SQ) is registered at runtime into
concourse.dve_ops.OPS, so this file is self-contained.

Sharding: data-parallel over batch, one image per NeuronCore (8 cores).
The output does not depend on the image *values*, so images never touch the
device; only tiny per-segment coefficient tables are uploaded.
"""

import numpy as np
from contextlib import ExitStack

B, C, H, W = 8, 3, 512, 512
K = 17
NSEG = K - 1
P = 128
NSTRIPE = H // P  # 4

# per-seg uniform scalar columns in `cs`
CS_DXS, CS_AP, CS_DN2S, CS_NDN2S = 0, 1, 2, 3
CS_PER_SEG = 4
CS_THR = CS_PER_SEG * NSEG
CS_COLS = CS_THR + 1

_state = {}


def _register_dve_op(name, spec):
    import concourse.dve_ops as dve_ops
    from concourse.dve_ops import DveOp, OPS, _SUB_OPCODE_FOR_NAME, _CUSTOM_DVE_ROW_BASE
    from concourse.dve_spec import lower, _has_src1
    from concourse.dve_uop import DveOpSpec
    from concourse.dve_table_gen import dve_ver_for

    if name in _SUB_OPCODE_FOR_NAME:
        return next(o for o in OPS if o.name == name)
    row = _CUSTOM_DVE_ROW_BASE + len(OPS)
    assert row < 0x20
    _SUB_OPCODE_FOR_NAME[name] = row
    ver = dve_ver_for("TRN2")
    tmp = DveOpSpec(
        name=name, opcode=row, uops=lower(spec, ver=ver), rd1_en=_has_src1(spec)
    )
    op = DveOp(name, spec, subdim=False, uops_sha={ver: tmp.sha(ver)})
    OPS.append(op)
    dve_ops.CUSTOM_DVE_SPECS[name] = spec
    return op


def _get_dve_ops():
    """Register (once) the two fused DVE ops:
      D2MIN: out = min((Idx*s0 + s1)^2 + Src0^2, Src1)
      D2:    out = (Idx*s0 + s1)^2 + Src0^2        (first segment, no min)
    Idx is the DVE free-dim index generator == the x coordinate, so the
    perpendicular-plane term costs no tensor input and Src1 can carry the
    running minimum."""
    if "ops" in _state:
        return _state["ops"]
    from concourse.dve_spec import Spec, Src0, Src1, C0, C1, sq, minn, Idx

    def _idx(in0):
        return np.arange(in0.shape[-1], dtype=np.float32)[None, :]

    d2min = _register_dve_op(
        "STROKE_D2MIN_ANT",
        Spec(
            body=minn(sq(Idx * C0 + C1) + sq(Src0), Src1),
            reference=lambda in0, in1, s0, s1, imm2: np.minimum(
                (_idx(in0) * s0 + s1) ** 2 + in0.astype(np.float32) ** 2, in1
            ).astype(np.float32),
        ),
    )
    d2first = _register_dve_op(
        "STROKE_D2_ANT",
        Spec(
            body=sq(Idx * C0 + C1) + sq(Src0),
            reference=lambda in0, in1, s0, s1, imm2: (
                (_idx(in0) * s0 + s1) ** 2 + in0.astype(np.float32) ** 2
            ).astype(np.float32),
        ),
    )
    _state["ops"] = (d2min, d2first)
    return _state["ops"]


def _build_program():
    import concourse.bass as bass
    import concourse.tile as tile
    from concourse import bacc, mybir

    dt = mybir.dt
    op = mybir.AluOpType
    af = mybir.ActivationFunctionType
    d2min_op, d2first_op = _get_dve_ops()

    nc = bacc.Bacc(
        "TRN2", target_bir_lowering=False, debug=False, num_devices=8
    )
    xt_d = nc.dram_tensor("xt", [P, W], dt.float32, kind="ExternalInput").ap()
    cs_d = nc.dram_tensor("cs", [P, CS_COLS], dt.float32, kind="ExternalInput").ap()
    cdw_d = nc.dram_tensor("cdw", [P, NSTRIPE * NSEG], dt.float32, kind="ExternalInput").ap()
    cbp_d = nc.dram_tensor("cbp", [P, NSTRIPE * NSEG], dt.float32, kind="ExternalInput").ap()
    out_d = nc.dram_tensor("out", [C, H, W], dt.float32, kind="ExternalOutput").ap()

    with tile.TileContext(nc) as tc, ExitStack() as ctx:
        const = ctx.enter_context(tc.tile_pool(name="const", bufs=1))
        xt = const.tile_from(xt_d)
        cs = const.tile_from(cs_d)
        cdw = const.tile_from(cdw_d)
        cbp = const.tile_from(cbp_d)

        work = ctx.enter_context(tc.tile_pool(name="work", bufs=4))
        mpool = ctx.enter_context(tc.tile_pool(name="m", bufs=3))
        opool = ctx.enter_context(tc.tile_pool(name="o", bufs=3))

        def seg_col(s, which):
            c = s * CS_PER_SEG + which
            return cs[:, c : c + 1]

        for T in range(NSTRIPE):
            M = None
            for s in range(NSEG):
                col = T * NSEG + s
                # At = |(dx*x + dy*y - c0 - dn2) * s|   [scalar ACT]
                At = work.tile([P, W], dt.float32, tag="At")
                nc.scalar.activation(
                    At[:], xt[:], af.Abs,
                    bias=cdw[:, col : col + 1], scale=seg_col(s, CS_DXS),
                )
                # E = relu(At - dn2s); split between V-ts and ACT-Relu to
                # balance engine load (V: custom op is 2 uOp passes; ACT: At).
                E = work.tile([P, W], dt.float32, tag="E")
                if s % 8 < 3:
                    nc.scalar.activation(
                        E[:], At[:], af.Relu, bias=seg_col(s, CS_NDN2S)
                    )
                else:
                    nc.vector.tensor_scalar(
                        E[:], At[:], seg_col(s, CS_DN2S), 0.0,
                        op0=op.subtract, op1=op.max,
                    )
                # M' = min((aP*x + bP)^2 + E^2, M)   [one fused custom DVE op]
                Mn = mpool.tile([P, W], dt.float32, tag="M")
                if s == 0:
                    nc.vector._custom_dve(
                        d2first_op, out=Mn[:], in0=E[:],
                        s0=seg_col(s, CS_AP), s1=cbp[:, col : col + 1],
                    )
                else:
                    nc.vector._custom_dve(
                        d2min_op, out=Mn[:], in0=E[:], in1=M[:],
                        s0=seg_col(s, CS_AP), s1=cbp[:, col : col + 1],
                    )
                M = Mn

            # dist = sqrt(M); cov = clip(thr - dist, 0, 1)
            dist = opool.tile([P, W], dt.float32, tag="dist")
            nc.scalar.activation(dist[:], M[:], af.Sqrt)
            cov1 = opool.tile([P, W], dt.float32, tag="cov1")
            nc.vector.tensor_scalar(
                cov1[:], dist[:], -1.0, cs[:, CS_THR : CS_THR + 1],
                op0=op.mult, op1=op.add,
            )
            cov = opool.tile([P, W], dt.float32, tag="cov")
            nc.vector.tensor_scalar(
                cov[:], cov1[:], 0.0, 1.0, op0=op.max, op1=op.min
            )
            for c in range(C):
                nc.sync.dma_start(out_d[c, T * P : (T + 1) * P, :], cov[:])

    nc.compile()
    return nc


def _prep_inputs(trajectories, line_width):
    """Host-side per-segment coefficient tables (numpy, float64 -> float32)."""
    thr = float(np.asarray(line_width).item()) + 0.5
    xt = np.broadcast_to(
        np.arange(W, dtype=np.float64), (P, W)
    ).astype(np.float32)
    xy = np.asarray(trajectories, dtype=np.float64)[:, :, 1:3]  # [B, K, 2]
    yv = np.arange(H, dtype=np.float64).reshape(NSTRIPE, P)  # y = T*128 + p

    in_maps = []
    for b in range(xy.shape[0]):
        p0, p1 = xy[b, :-1], xy[b, 1:]
        d = p1 - p0
        # degenerate-segment guard (measure-zero with random inputs)
        degen = (d[:, 0] ** 2 + d[:, 1] ** 2) < 1e-12
        d[degen, 0] = 1e-6
        dx, dy = d[:, 0], d[:, 1]
        p0x, p0y = p0[:, 0], p0[:, 1]
        dd2 = dx * dx + dy * dy
        sq = 1.0 / np.sqrt(dd2)
        dn2 = dd2 / 2.0
        c0 = dx * p0x + dy * p0y
        cP = dx * p0y - dy * p0x

        # [T, p, s] -> [p, T*NSEG+s]
        cdw = (dy[None, None, :] * yv[:, :, None] - (c0 + dn2)[None, None, :]) * sq[None, None, :]
        cdw = np.transpose(cdw, (1, 0, 2)).reshape(P, NSTRIPE * NSEG)
        cbp = (-dx[None, None, :] * yv[:, :, None] + cP[None, None, :]) * sq[None, None, :]
        cbp = np.transpose(cbp, (1, 0, 2)).reshape(P, NSTRIPE * NSEG)

        cs = np.zeros((P, CS_COLS), dtype=np.float64)
        cs[:, CS_DXS : CS_PER_SEG * NSEG : CS_PER_SEG] = dx * sq
        cs[:, CS_AP : CS_PER_SEG * NSEG : CS_PER_SEG] = dy * sq
        cs[:, CS_DN2S : CS_PER_SEG * NSEG : CS_PER_SEG] = dn2 * sq
        cs[:, CS_NDN2S : CS_PER_SEG * NSEG : CS_PER_SEG] = -dn2 * sq
        cs[:, CS_THR] = thr

        in_maps.append(
            {
                "xt": xt,
                "cs": cs.astype(np.float32),
                "cdw": cdw.astype(np.float32),
                "cbp": cbp.astype(np.float32),
            }
        )
    return in_maps


def kernel(**inputs):
    from concourse.bass_utils import run_bass_kernel_spmd

    images = np.asarray(inputs["images"])
    trajectories = np.asarray(inputs["trajectories"])
    line_width = inputs["line_width"]
    assert images.shape == (B, C, H, W), images.shape

    if "nc" not in _state:
        _state["nc"] = _build_program()
    nc = _state["nc"]

    in_maps = _prep_inputs(trajectories, line_width)
    res = run_bass_kernel_spmd(nc, in_maps, list(range(B))).results
    out = np.stack([res[i]["out"] for i in range(B)], axis=0)
    return out.astype(np.float32)


if __name__ == "__main__":
    rng = np.random.default_rng(0)
    ins = {
        "images": rng.standard_normal((B, C, H, W)).astype(np.float32),
        "trajectories": np.concatenate(
            [
                np.broadcast_to(np.linspace(0, 1, K, dtype=np.float32), (B, K))[..., None],
                rng.uniform(0, W - 1, (B, K, 2)).astype(np.float32),
                np.ones((B, K, 1), np.float32),
            ],
            axis=-1,
        ),
        "line_width": 3,
    }
    out = kernel(**ins)
    print(out.shape, out.dtype, out.min(), out.max())
